# revision 12
# baseline (speedup 1.0000x reference)
"""Trainium2 Bass kernel for nn_AttentionBlock (B=8, C=512, H=W=64, GN32 + QKV + attention + proj + residual).

Sharding: data-parallel over batch across the 8 NeuronCores (one sample per core).

Per-core pipeline (sample x_b [512, 4096]):
  A. GroupNorm stats: per-channel mean/var via bn_stats, group-reduce via
     mask matmuls, broadcast back to per-channel affine (rs, tb).
  B. xn = rs*x + tb; q = Wq' xn + bq (scale folded), k likewise, vT = xn^T Wv' + bv.
     q/k/vT stored bf16 (k, vT stay in SBUF; q spills to HBM).
  C. Flash-style attention per 512-wide query block:
       S^T[m,nb] = k^T q   (bf16 matmuls, PSUM fp32)
       P^T = exp(S^T)      (ScalarE, no max subtraction: |S| <~ 1.2)
       O  += vT^T P^T, rowsum += ones^T P^T   (accumulated over m-tiles)
       hf = O * (1/rowsum);  out = Wp^T hf + pb + x   (fp32r matmuls)

gn gamma/beta and the qk scale C^-0.25 are folded into qkv_w/qkv_b on host.
"""

import numpy as np

import concourse.bass as bass
import concourse.tile as tile
from concourse import bacc, mybir
from concourse import bass_utils

# Problem dims (hardcoded per contest rules)
B = 8
C = 512
H = W = 64
N = H * W          # 4096
G = 32             # groups
CPG = C // G       # 16 channels per group
EPS = 1e-5
P = 128
CCH = C // P       # 4 channel chunks
NBS = 8            # n blocks
NBW = N // NBS     # 512 block width
MTS = N // P       # 32 m tiles

FP32 = mybir.dt.float32
FP32R = mybir.dt.float32r
BF16 = mybir.dt.bfloat16

_CACHED = None  # (nc, input_names)


def r(ap, dt=FP32R):
    return ap.bitcast(dt)


def build_program():
    nc = bacc.Bacc("TRN2", target_bir_lowering=False, debug=False, num_devices=B)

    x = nc.dram_tensor("x", [C, N], FP32, kind="ExternalInput").ap()
    wqkvT = nc.dram_tensor("wqkvT", [P, CCH, 3 * C], FP32R, kind="ExternalInput").ap()
    wpT = nc.dram_tensor("wpT", [P, CCH, C], FP32R, kind="ExternalInput").ap()
    qb = nc.dram_tensor("qb", [P, CCH], FP32, kind="ExternalInput").ap()
    kb = nc.dram_tensor("kb", [P, CCH], FP32, kind="ExternalInput").ap()
    vb = nc.dram_tensor("vb", [P, C], FP32, kind="ExternalInput").ap()
    pb = nc.dram_tensor("pb", [P, CCH], FP32, kind="ExternalInput").ap()
    maskT = nc.dram_tensor("maskT", [P, CCH, G], FP32, kind="ExternalInput").ap()
    maskB = nc.dram_tensor("maskB", [P, CCH, P], FP32, kind="ExternalInput").ap()
    out = nc.dram_tensor("out", [C, N], FP32, kind="ExternalOutput").ap()

    with tile.TileContext(nc) as tc:
        _body(nc, tc, x, wqkvT, wpT, qb, kb, vb, pb, maskT, maskB, out)
    nc.compile()
    return nc


def _body(nc, tc, x, wqkvT, wpT, qb, kb, vb, pb, maskT, maskB, out):
    FT = mybir.ActivationFunctionType
    OP = mybir.AluOpType

    with (
        tc.tile_pool(name="singles", bufs=1) as singles,
        tc.tile_pool(name="kv", bufs=1) as kv_pool,
        tc.tile_pool(name="dram", bufs=1, space="DRAM") as dram,
    ):
        # ---- constants / weights resident in SBUF ----
        wq_sb = singles.tile([P, CCH, 3 * C], FP32R)
        nc.sync.dma_start(wq_sb[:], wqkvT[:])
        wp_sb = singles.tile([P, CCH, C], FP32R)
        nc.sync.dma_start(wp_sb[:], wpT[:])
        qb_sb = singles.tile([P, CCH], FP32)
        nc.sync.dma_start(qb_sb[:], qb[:])
        kb_sb = singles.tile([P, CCH], FP32)
        nc.sync.dma_start(kb_sb[:], kb[:])
        vb_sb = singles.tile([P, C], FP32)
        nc.sync.dma_start(vb_sb[:], vb[:])
        pb_sb = singles.tile([P, CCH], FP32)
        nc.sync.dma_start(pb_sb[:], pb[:])
        maskT_sb = singles.tile([P, CCH, G], FP32)
        nc.sync.dma_start(maskT_sb[:], maskT[:])
        maskB_sb = singles.tile([P, CCH, P], FP32)
        nc.sync.dma_start(maskB_sb[:], maskB[:])
        ones_sb = singles.tile([P, P], BF16)
        nc.vector.memset(ones_sb[:], 1.0)
        eps_sb = singles.tile([P, 1], FP32)
        nc.vector.memset(eps_sb[:], EPS)

        # persistent bf16 activations
        k_sb = kv_pool.tile([P, CCH, N], BF16)     # [c'chunkP, cc, m]
        vT_sb = kv_pool.tile([P, MTS, C], BF16)    # [mP, mt, c]
        q_dram = dram.tile([CCH, P, N], BF16)      # q spilled to HBM

        with tc.tile_pool(name="xnp", bufs=1) as xn_pool:
            xn_t = []
            with (
                tc.tile_pool(name="xin", bufs=2) as x_pool,
                tc.tile_pool(name="stat", bufs=2) as stat_pool,
                tc.tile_pool(name="gsmall", bufs=1) as gs_pool,
                tc.tile_pool(name="pstat", bufs=2, space="PSUM") as pstat_pool,
            ):
                # ---- Phase A: group norm statistics ----
                rhs_stats = gs_pool.tile([P, CCH, 2], FP32)
                for cc in range(CCH):
                    xt = x_pool.tile([P, N], FP32, name="x_t")
                    nc.sync.dma_start(xt[:], x[cc * P:(cc + 1) * P, :])
                    stats = stat_pool.tile([P, N // 512, 6], FP32, name="bst")
                    xt_g = xt[:].rearrange("p (s f) -> p s f", f=512)
                    for s in range(N // 512):
                        nc.vector.bn_stats(out=stats[:, s, :], in_=xt_g[:, s, :])
                    mv = stat_pool.tile([P, 2], FP32, name="mv")
                    nc.vector.bn_aggr(out=mv[:], in_=stats[:])
                    # rhs_stats[:, cc, 0] = mean_c ; [:, cc, 1] = var_c + mean_c^2
                    nc.vector.tensor_copy(out=rhs_stats[:, cc, 0:1], in_=mv[:, 0:1])
                    nc.vector.tensor_tensor(
                        rhs_stats[:, cc, 1:2], mv[:, 0:1], mv[:, 0:1], OP.mult
                    )
                    nc.vector.tensor_add(
                        rhs_stats[:, cc, 1:2], rhs_stats[:, cc, 1:2], mv[:, 1:2]
                    )

                # group reduce: [32, 2] = sum_cc maskT[:,cc,:].T @ rhs_stats[:,cc,:]
                gps = pstat_pool.tile([G, 2], FP32, name="gps")
                for cc in range(CCH):
                    nc.tensor.matmul(
                        gps[:], maskT_sb[:, cc, :], rhs_stats[:, cc, :],
                        start=(cc == 0), stop=(cc == CCH - 1),
                    )
                gstats = gs_pool.tile([G, 2], FP32)
                nc.vector.tensor_copy(out=gstats[:], in_=gps[:])

                # rs = rsqrt(var_g + eps); tb = -mean_g * rs   (in ab_g [128,2], g-padded)
                ab_g = gs_pool.tile([P, 2], FP32)
                nc.vector.memset(ab_g[:], 0.0)
                gvar = gs_pool.tile([G, 1], FP32)
                nc.vector.tensor_tensor(gvar[:], gstats[:, 0:1], gstats[:, 0:1], OP.mult)
                nc.vector.tensor_sub(gvar[:], gstats[:, 1:2], gvar[:])
                nc.scalar.activation(
                    out=gvar[:], in_=gvar[:], func=FT.Sqrt, bias=eps_sb[0:G, :]
                )
                nc.vector.reciprocal(out=ab_g[0:G, 0:1], in_=gvar[:])
                nc.vector.tensor_tensor(
                    ab_g[0:G, 1:2], gstats[:, 0:1], ab_g[0:G, 0:1], OP.mult
                )
                nc.vector.tensor_scalar_mul(ab_g[0:G, 1:2], ab_g[0:G, 1:2], -1.0)

                # broadcast to channels: ab_c[:, cc, :] = maskB[:,cc,:].T @ ab_g
                ab_c = gs_pool.tile([P, CCH, 2], FP32)
                for cc in range(CCH):
                    abps = pstat_pool.tile([P, 2], FP32, name="abps")
                    nc.tensor.matmul(
                        abps[:], maskB_sb[:, cc, :], ab_g[:], start=True, stop=True
                    )
                    nc.vector.tensor_copy(out=ab_c[:, cc, :], in_=abps[:])

                # ---- xn = rs*x + tb  (x re-streamed; fp32r out so the PE
                # consumes it at full rate) ----
                for cc in range(CCH):
                    xt = x_pool.tile([P, N], FP32, name="x_t")
                    nc.sync.dma_start(xt[:], x[cc * P:(cc + 1) * P, :])
                    xn = xn_pool.tile([P, N], FP32R, name=f"xn_{cc}")
                    nc.vector.tensor_scalar(
                        out=xn[:], in0=xt[:],
                        scalar1=ab_c[:, cc, 0:1], scalar2=ab_c[:, cc, 1:2],
                        op0=OP.mult, op1=OP.add,
                    )
                    xn_t.append(xn)

            # ---- Phase B: qkv projections ----
            with (
                tc.tile_pool(name="qkst", bufs=4) as st_pool,
                tc.tile_pool(name="pqkv", bufs=3, space="PSUM") as pqkv,
            ):
                for oc in range(CCH):
                    for nb in range(NBS):
                        nsl = bass.ts(nb, NBW)
                        # q
                        ps = pqkv.tile([P, NBW], FP32, name="qk_ps")
                        for cc in range(CCH):
                            nc.tensor.matmul(
                                ps[:],
                                wq_sb[:, cc, oc * P:(oc + 1) * P],
                                r(xn_t[cc][:, nsl]),
                                start=(cc == 0), stop=(cc == CCH - 1),
                            )
                        qst = st_pool.tile([P, NBW], BF16, name="q_st")
                        nc.vector.tensor_scalar_add(qst[:], ps[:], qb_sb[:, oc:oc + 1])
                        nc.sync.dma_start(q_dram[oc, :, nsl], qst[:])
                        # k (bias via ScalarE to balance engines)
                        ps2 = pqkv.tile([P, NBW], FP32, name="qk_ps")
                        for cc in range(CCH):
                            nc.tensor.matmul(
                                ps2[:],
                                wq_sb[:, cc, C + oc * P:C + (oc + 1) * P],
                                r(xn_t[cc][:, nsl]),
                                start=(cc == 0), stop=(cc == CCH - 1),
                            )
                        nc.scalar.activation(
                            out=k_sb[:, oc, nsl], in_=ps2[:],
                            func=FT.Identity, bias=kb_sb[:, oc:oc + 1],
                        )
                # vT
                for mt in range(MTS):
                    msl = bass.ts(mt, P)
                    ps = pqkv.tile([P, C], FP32, name="qk_ps")
                    for cc in range(CCH):
                        nc.tensor.matmul(
                            ps[:],
                            r(xn_t[cc][:, msl]),
                            wq_sb[:, cc, 2 * C:3 * C],
                            start=(cc == 0), stop=(cc == CCH - 1),
                        )
                    nc.vector.tensor_add(vT_sb[:, mt, :], ps[:], vb_sb[:])

        # ---- Phase C: attention + proj + residual ----
        with (
            tc.tile_pool(name="qt", bufs=2) as qt_pool,
            tc.tile_pool(name="pt", bufs=3) as pt_pool,
            tc.tile_pool(name="hf", bufs=2) as hf_pool,
            tc.tile_pool(name="xr", bufs=2) as xr_pool,
            tc.tile_pool(name="ost", bufs=4) as ost_pool,
            tc.tile_pool(name="pacc", bufs=1, space="PSUM") as pacc,
            tc.tile_pool(name="psmall", bufs=2, space="PSUM") as psm,
        ):
            for nb in range(NBS):
                nsl = bass.ts(nb, NBW)
                q_t = qt_pool.tile([P, CCH, NBW], BF16, name="q_t")
                nc.sync.dma_start(
                    q_t[:], q_dram[:, :, nsl].rearrange("a p n -> p a n")
                )
                xr = xr_pool.tile([P, CCH, NBW], FP32, name="xr")
                for oc in range(CCH):
                    nc.sync.dma_start(
                        xr[:, oc, :], x[oc * P:(oc + 1) * P, nsl]
                    )

                o_ps = pacc.tile([P, CCH, NBW], FP32, name="o_ps")
                r_ps = pacc.tile([P, NBW], FP32, name="r_ps")
                for mt in range(MTS):
                    msl = bass.ts(mt, P)
                    s_ps = psm.tile([P, NBW], FP32, name="s_ps")
                    for cc in range(CCH):
                        nc.tensor.matmul(
                            s_ps[:], k_sb[:, cc, msl], q_t[:, cc, :],
                            start=(cc == 0), stop=(cc == CCH - 1),
                        )
                    pt = pt_pool.tile([P, NBW], BF16, name="pt")
                    nc.scalar.activation(out=pt[:], in_=s_ps[:], func=FT.Exp)
                    for oc in range(CCH):
                        nc.tensor.matmul(
                            o_ps[:, oc, :], vT_sb[:, mt, oc * P:(oc + 1) * P], pt[:],
                            start=(mt == 0), stop=(mt == MTS - 1),
                        )
                    nc.tensor.matmul(
                        r_ps[:], ones_sb[:], pt[:],
                        start=(mt == 0), stop=(mt == MTS - 1),
                    )

                inv = hf_pool.tile([P, NBW], FP32, name="inv")
                nc.vector.reciprocal(out=inv[:], in_=r_ps[:])
                hf = hf_pool.tile([P, CCH, NBW], FP32R, name="hf")
                for oc in range(CCH):
                    nc.vector.tensor_mul(hf[:, oc, :], o_ps[:, oc, :], inv[:])

                for oc in range(CCH):
                    pr_ps = psm.tile([P, NBW], FP32, name="s_ps")
                    for cc in range(CCH):
                        nc.tensor.matmul(
                            pr_ps[:],
                            wp_sb[:, cc, oc * P:(oc + 1) * P],
                            hf[:, cc, :],
                            start=(cc == 0), stop=(cc == CCH - 1),
                        )
                    o_t = ost_pool.tile([P, NBW], FP32, name="o_t")
                    nc.vector.scalar_tensor_tensor(
                        out=o_t[:], in0=pr_ps[:], scalar=pb_sb[:, oc:oc + 1],
                        in1=xr[:, oc, :], op0=OP.add, op1=OP.add,
                    )
                    nc.sync.dma_start(out[oc * P:(oc + 1) * P, nsl], o_t[:])


def _host_prep(inputs):
    x = np.ascontiguousarray(np.asarray(inputs["x"], dtype=np.float32))
    gamma = np.asarray(inputs["gn_gamma"], dtype=np.float32)
    beta = np.asarray(inputs["gn_beta"], dtype=np.float32)
    qkv_w = np.asarray(inputs["qkv_w"], dtype=np.float32)
    qkv_b = np.asarray(inputs["qkv_b"], dtype=np.float32)
    proj_w = np.asarray(inputs["proj_w"], dtype=np.float32)
    proj_b = np.asarray(inputs["proj_b"], dtype=np.float32)

    scale = float(C) ** -0.25
    Wf = qkv_w * gamma[None, :]
    bf = qkv_b + qkv_w @ beta
    Wf = Wf.copy()
    Wf[: 2 * C] *= scale
    bf = bf.copy()
    bf[: 2 * C] *= scale

    wqkvT = np.ascontiguousarray(
        Wf.T.reshape(CCH, P, 3 * C).transpose(1, 0, 2), dtype=np.float32
    )  # [P, CCH, 3C]
    wpT = np.ascontiguousarray(
        proj_w.T.reshape(CCH, P, C).transpose(1, 0, 2), dtype=np.float32
    )
    qb = np.ascontiguousarray(bf[0:C].reshape(CCH, P).T, dtype=np.float32)
    kb = np.ascontiguousarray(bf[C:2 * C].reshape(CCH, P).T, dtype=np.float32)
    vb = np.ascontiguousarray(np.tile(bf[2 * C:3 * C][None, :], (P, 1)), dtype=np.float32)
    pbh = np.ascontiguousarray(proj_b.reshape(CCH, P).T, dtype=np.float32)

    chan = np.arange(C)
    grp = chan // CPG
    maskT = np.zeros((C, G), np.float32)
    maskT[chan, grp] = 1.0 / CPG
    maskT = np.ascontiguousarray(maskT.reshape(CCH, P, G).transpose(1, 0, 2))
    maskB = np.zeros((P, CCH, P), np.float32)
    for cc in range(CCH):
        for p in range(P):
            maskB[grp[cc * P + p], cc, p] = 1.0

    shared = dict(
        wqkvT=wqkvT, wpT=wpT, qb=qb, kb=kb, vb=vb, pb=pbh,
        maskT=maskT, maskB=maskB,
    )
    in_maps = []
    for b in range(B):
        m = dict(shared)
        m["x"] = np.ascontiguousarray(x[b].reshape(C, N))
        in_maps.append(m)
    return in_maps


def _get_program():
    global _CACHED
    if _CACHED is None:
        _CACHED = build_program()
    return _CACHED


def run(inputs, trace=False, **kw):
    nc = _get_program()
    in_maps = _host_prep(inputs)
    res = bass_utils.run_bass_kernel_spmd(
        nc, in_maps, core_ids=list(range(B)), trace=trace, **kw
    )
    outs = np.stack([res.results[b]["out"].reshape(C, H, W) for b in range(B)])
    return outs.astype(np.float32), res


def kernel(**inputs) -> np.ndarray:
    outs, _ = run(inputs, trace=False)
    return outs


if __name__ == "__main__":
    import reference

    inputs = reference.setup_inputs()
    outs, res = run({k: np.asarray(v) for k, v in inputs.items()}, trace=True)
    print("exec_time_ns:", res.exec_time_ns)


# revision 13
# speedup vs baseline: 1.6352x; 1.6352x over previous
"""Trainium2 Bass kernel for nn_AttentionBlock (B=8, C=512, H=W=64, GN32 + QKV + attention + proj + residual).

Sharding: data-parallel over batch across the 8 NeuronCores (one sample per core).

Per-core pipeline (sample x_b [512, 4096]):
  A. GroupNorm stats: per-channel sum (DVE reduce) and sumsq (ScalarE Square
     with accumulate), group-reduce + per-channel broadcast via mask matmuls.
  B. xn = rs*x + tb (fp8); q/k/vT = 1x1 convs as fp8 DoubleRow matmuls
     (gn gamma/beta and the qk scale C^-0.25 folded into weights on host).
  C. Flash-style attention per 512-wide query block, all fp8 DoubleRow:
       S^T[m,nb] = k^T q ;  P^T = exp(S^T)  (no max subtraction: |S| <~ 1.2)
       O += vT^T P^T ; rowsum += ones^T P^T  (accumulated over m-tile pairs)
       proj on unnormalized O (fp32r), then out = (Wp O)*inv + pb + x
     (softmax normalization commutes with the 1x1 conv, so the reciprocal
      stays off the PE critical path).
"""

import numpy as np
import ml_dtypes

import concourse.bass as bass
import concourse.tile as tile
from concourse import bacc, mybir
from concourse import bass_utils

B = 8
C = 512
H = W = 64
N = H * W          # 4096
G = 32             # groups
CPG = C // G       # 16 channels per group
EPS = 1e-5
P = 128
CCH = C // P       # 4 channel chunks
NBS = 8            # n blocks
NBW = N // NBS     # 512 block width
MTS = N // P       # 32 m tiles

FP32 = mybir.dt.float32
FP32R = mybir.dt.float32r
FP8 = mybir.dt.float8e4

_CACHED = None


def build_program():
    nc = bacc.Bacc("TRN2", target_bir_lowering=False, debug=False, num_devices=B)

    x = nc.dram_tensor("x", [C, N], FP32, kind="ExternalInput").ap()
    wqkv8 = nc.dram_tensor("wqkv8", [P, CCH, 3 * C], FP8, kind="ExternalInput").ap()
    wpT = nc.dram_tensor("wpT", [P, CCH, C], FP32R, kind="ExternalInput").ap()
    qb = nc.dram_tensor("qb", [P, CCH], FP32, kind="ExternalInput").ap()
    kb = nc.dram_tensor("kb", [P, CCH], FP32, kind="ExternalInput").ap()
    vb = nc.dram_tensor("vb", [P, C], FP32, kind="ExternalInput").ap()
    pb = nc.dram_tensor("pb", [P, CCH], FP32, kind="ExternalInput").ap()
    maskT = nc.dram_tensor("maskT", [P, CCH, G], FP32, kind="ExternalInput").ap()
    maskB = nc.dram_tensor("maskB", [P, CCH, P], FP32, kind="ExternalInput").ap()
    out = nc.dram_tensor("out", [C, N], FP32, kind="ExternalOutput").ap()

    with tile.TileContext(nc) as tc:
        _body(nc, tc, x, wqkv8, wpT, qb, kb, vb, pb, maskT, maskB, out)
    nc.compile()
    return nc


def _body(nc, tc, x, wqkv8, wpT, qb, kb, vb, pb, maskT, maskB, out):
    FT = mybir.ActivationFunctionType
    OP = mybir.AluOpType
    DR = mybir.MatmulPerfMode.DoubleRow

    with (
        tc.tile_pool(name="singles", bufs=1) as singles,
        tc.tile_pool(name="acts", bufs=1) as acts,
    ):
        # ---- constants / weights resident in SBUF ----
        wq_sb = singles.tile([P, CCH, 3 * C], FP8)
        nc.sync.dma_start(wq_sb[:], wqkv8[:])
        wp_sb = singles.tile([P, CCH, C], FP32R)
        nc.sync.dma_start(wp_sb[:], wpT[:])
        qb_sb = singles.tile([P, CCH], FP32)
        nc.sync.dma_start(qb_sb[:], qb[:])
        kb_sb = singles.tile([P, CCH], FP32)
        nc.sync.dma_start(kb_sb[:], kb[:])
        vb_sb = singles.tile([P, C], FP32)
        nc.sync.dma_start(vb_sb[:], vb[:])
        pb_sb = singles.tile([P, CCH], FP32)
        nc.sync.dma_start(pb_sb[:], pb[:])
        maskT_sb = singles.tile([P, CCH, G], FP32)
        nc.sync.dma_start(maskT_sb[:], maskT[:])
        maskB_sb = singles.tile([P, CCH, P], FP32)
        nc.sync.dma_start(maskB_sb[:], maskB[:])
        ones_sb = singles.tile([P, 2, P], FP8)
        nc.vector.memset(ones_sb[:], 1.0)
        eps_sb = singles.tile([P, 1], FP32)
        nc.vector.memset(eps_sb[:], EPS)

        # persistent activations
        x_t = [acts.tile([P, N], FP32, name=f"x_{cc}") for cc in range(CCH)]
        xn8 = acts.tile([P, CCH, N], FP8)    # normalized input
        q8 = acts.tile([P, CCH, N], FP8)     # [c'P, c'chunk, n]
        k8 = acts.tile([P, CCH, N], FP8)     # [c'P, c'chunk, m]
        vT8 = acts.tile([P, MTS, C], FP8)    # [mP, mt, c]

        with (
            tc.tile_pool(name="stat", bufs=2) as stat_pool,
            tc.tile_pool(name="gsmall", bufs=1) as gs_pool,
            tc.tile_pool(name="pstat", bufs=2, space="PSUM") as pstat_pool,
        ):
            # ---- Phase A: group norm statistics ----
            # per-channel sum via DVE reduce; sumsq via ScalarE Square+accum
            # (Square main output scratched into q8, overwritten in phase B).
            rhs_stats = gs_pool.tile([P, CCH, 2], FP32)
            for cc in range(CCH):
                nc.sync.dma_start(x_t[cc][:], x[cc * P:(cc + 1) * P, :])
                nc.vector.reduce_sum(
                    rhs_stats[:, cc, 0:1], x_t[cc][:], axis=mybir.AxisListType.X
                )
                nc.scalar.activation(
                    out=q8[:, cc, :], in_=x_t[cc][:], func=FT.Square,
                    accum_out=rhs_stats[:, cc, 1:2],
                )

            # group reduce (maskT carries 1/(CPG*N)): [32,2] = [mean_g, Ex2_g]
            gps = pstat_pool.tile([G, 2], FP32, name="gps")
            for cc in range(CCH):
                nc.tensor.matmul(
                    gps[:], maskT_sb[:, cc, :], rhs_stats[:, cc, :],
                    start=(cc == 0), stop=(cc == CCH - 1),
                )
            gstats = gs_pool.tile([G, 2], FP32)
            nc.vector.tensor_copy(out=gstats[:], in_=gps[:])

            # rs = rsqrt(var_g + eps); tb = -mean_g * rs   (g-padded [128,2])
            ab_g = gs_pool.tile([P, 2], FP32)
            nc.vector.memset(ab_g[:], 0.0)
            gvar = gs_pool.tile([G, 1], FP32)
            nc.vector.tensor_tensor(gvar[:], gstats[:, 0:1], gstats[:, 0:1], OP.mult)
            nc.vector.tensor_sub(gvar[:], gstats[:, 1:2], gvar[:])
            nc.scalar.activation(
                out=gvar[:], in_=gvar[:], func=FT.Sqrt, bias=eps_sb[0:G, :]
            )
            nc.vector.reciprocal(out=ab_g[0:G, 0:1], in_=gvar[:])
            nc.vector.tensor_tensor(
                ab_g[0:G, 1:2], gstats[:, 0:1], ab_g[0:G, 0:1], OP.mult
            )
            nc.vector.tensor_scalar_mul(ab_g[0:G, 1:2], ab_g[0:G, 1:2], -1.0)

            # broadcast to channels: ab_c[:, cc, :] = maskB[:,cc,:].T @ ab_g
            ab_c = gs_pool.tile([P, CCH, 2], FP32)
            for cc in range(CCH):
                abps = pstat_pool.tile([P, 2], FP32, name="abps")
                nc.tensor.matmul(
                    abps[:], maskB_sb[:, cc, :], ab_g[:], start=True, stop=True
                )
                nc.vector.tensor_copy(out=ab_c[:, cc, :], in_=abps[:])

            # ---- xn = rs*x + tb -> fp8 (DVE/ACT split) ----
            for cc in range(CCH):
                if cc % 2 == 0:
                    nc.vector.tensor_scalar(
                        out=xn8[:, cc, :], in0=x_t[cc][:],
                        scalar1=ab_c[:, cc, 0:1], scalar2=ab_c[:, cc, 1:2],
                        op0=OP.mult, op1=OP.add,
                    )
                else:
                    nc.scalar.activation(
                        out=xn8[:, cc, :], in_=x_t[cc][:], func=FT.Identity,
                        bias=ab_c[:, cc, 1:2], scale=ab_c[:, cc, 0:1],
                    )

        # ---- Phase B: qkv projections (fp8 DoubleRow, K=256 per matmul) ----
        with tc.tile_pool(name="pqkv", bufs=3, space="PSUM") as pqkv:
            for oc in range(CCH):
                osl = bass.ts(oc, P)
                for nb in range(NBS):
                    nsl = bass.ts(nb, NBW)
                    ps = pqkv.tile([P, NBW], FP32, name="qk_ps")
                    for h in range(2):
                        nc.tensor.matmul(
                            ps[:],
                            wq_sb[:, 2 * h:2 * h + 2, osl],
                            xn8[:, 2 * h:2 * h + 2, nsl],
                            start=(h == 0), stop=(h == 1), perf_mode=DR,
                        )
                    nc.vector.tensor_scalar_add(
                        q8[:, oc, nsl], ps[:], qb_sb[:, oc:oc + 1]
                    )
                    ps2 = pqkv.tile([P, NBW], FP32, name="qk_ps")
                    for h in range(2):
                        nc.tensor.matmul(
                            ps2[:],
                            wq_sb[:, 2 * h:2 * h + 2, C + oc * P:C + (oc + 1) * P],
                            xn8[:, 2 * h:2 * h + 2, nsl],
                            start=(h == 0), stop=(h == 1), perf_mode=DR,
                        )
                    nc.scalar.activation(
                        out=k8[:, oc, nsl], in_=ps2[:],
                        func=FT.Identity, bias=kb_sb[:, oc:oc + 1],
                    )
            for mt in range(MTS):
                msl = bass.ts(mt, P)
                ps = pqkv.tile([P, C], FP32, name="qk_ps")
                for h in range(2):
                    nc.tensor.matmul(
                        ps[:],
                        xn8[:, 2 * h:2 * h + 2, msl],
                        wq_sb[:, 2 * h:2 * h + 2, 2 * C:3 * C],
                        start=(h == 0), stop=(h == 1), perf_mode=DR,
                    )
                nc.vector.tensor_add(vT8[:, mt, :], ps[:], vb_sb[:])

        # ---- Phase C: attention + proj + residual ----
        with (
            tc.tile_pool(name="pt", bufs=3) as pt_pool,
            tc.tile_pool(name="hf", bufs=2) as hf_pool,
            tc.tile_pool(name="ost", bufs=4) as ost_pool,
            tc.tile_pool(name="pacc", bufs=1, space="PSUM") as pacc,
            tc.tile_pool(name="psmall", bufs=2, space="PSUM") as psm,
            tc.tile_pool(name="pproj", bufs=1, space="PSUM") as ppr,
        ):
            for nb in range(NBS):
                nsl = bass.ts(nb, NBW)
                o_ps = pacc.tile([P, CCH, NBW], FP32, name="o_ps")
                r_ps = pacc.tile([P, NBW], FP32, name="r_ps")
                for u in range(MTS // 2):
                    pt = pt_pool.tile([P, 2, NBW], FP8, name="pt")
                    for j in range(2):
                        mt = 2 * u + j
                        msl = bass.ts(mt, P)
                        s_ps = psm.tile([P, NBW], FP32, name="s_ps")
                        for h in range(2):
                            nc.tensor.matmul(
                                s_ps[:],
                                k8[:, 2 * h:2 * h + 2, msl],
                                q8[:, 2 * h:2 * h + 2, nsl],
                                start=(h == 0), stop=(h == 1), perf_mode=DR,
                            )
                        nc.scalar.activation(out=pt[:, j, :], in_=s_ps[:], func=FT.Exp)
                    for oc in range(CCH):
                        nc.tensor.matmul(
                            o_ps[:, oc, :],
                            vT8[:, 2 * u:2 * u + 2, bass.ts(oc, P)],
                            pt[:],
                            start=(u == 0), stop=(u == MTS // 2 - 1), perf_mode=DR,
                        )
                    nc.tensor.matmul(
                        r_ps[:], ones_sb[:], pt[:],
                        start=(u == 0), stop=(u == MTS // 2 - 1), perf_mode=DR,
                    )

                # unnormalized O -> fp32r; proj immediately (PE keeps streaming);
                # softmax denominator applied after proj (it commutes).
                inv = hf_pool.tile([P, NBW], FP32, name="inv")
                hfr = hf_pool.tile([P, CCH, NBW], FP32R, name="hfr")
                for oc in range(CCH):
                    nc.vector.tensor_copy(out=hfr[:, oc, :], in_=o_ps[:, oc, :])
                nc.vector.reciprocal(out=inv[:], in_=r_ps[:])
                for oc in range(CCH):
                    pr_ps = ppr.tile([P, NBW], FP32, name="pr_ps")
                    for cc in range(CCH):
                        nc.tensor.matmul(
                            pr_ps[:],
                            wp_sb[:, cc, bass.ts(oc, P)],
                            hfr[:, cc, :],
                            start=(cc == 0), stop=(cc == CCH - 1),
                        )
                    o_t = ost_pool.tile([P, NBW], FP32, name="o_t")
                    nc.vector.tensor_tensor(o_t[:], pr_ps[:], inv[:], OP.mult)
                    nc.vector.scalar_tensor_tensor(
                        out=o_t[:], in0=o_t[:], scalar=pb_sb[:, oc:oc + 1],
                        in1=x_t[oc][:, nsl], op0=OP.add, op1=OP.add,
                    )
                    nc.sync.dma_start(out[oc * P:(oc + 1) * P, nsl], o_t[:])


def _host_prep(inputs):
    x = np.ascontiguousarray(np.asarray(inputs["x"], dtype=np.float32))
    gamma = np.asarray(inputs["gn_gamma"], dtype=np.float32)
    beta = np.asarray(inputs["gn_beta"], dtype=np.float32)
    qkv_w = np.asarray(inputs["qkv_w"], dtype=np.float32)
    qkv_b = np.asarray(inputs["qkv_b"], dtype=np.float32)
    proj_w = np.asarray(inputs["proj_w"], dtype=np.float32)
    proj_b = np.asarray(inputs["proj_b"], dtype=np.float32)

    scale = float(C) ** -0.25
    Wf = (qkv_w * gamma[None, :]).copy()
    bf = (qkv_b + qkv_w @ beta).copy()
    Wf[: 2 * C] *= scale
    bf[: 2 * C] *= scale

    wqkv8 = np.ascontiguousarray(
        Wf.T.reshape(CCH, P, 3 * C).transpose(1, 0, 2)
    ).astype(ml_dtypes.float8_e4m3fn)
    wpT = np.ascontiguousarray(
        proj_w.T.reshape(CCH, P, C).transpose(1, 0, 2), dtype=np.float32
    )
    qb = np.ascontiguousarray(bf[0:C].reshape(CCH, P).T, dtype=np.float32)
    kb = np.ascontiguousarray(bf[C:2 * C].reshape(CCH, P).T, dtype=np.float32)
    vb = np.ascontiguousarray(np.tile(bf[2 * C:3 * C][None, :], (P, 1)), dtype=np.float32)
    pbh = np.ascontiguousarray(proj_b.reshape(CCH, P).T, dtype=np.float32)

    chan = np.arange(C)
    grp = chan // CPG
    maskTh = np.zeros((C, G), np.float32)
    maskTh[chan, grp] = 1.0 / (CPG * N)
    maskTh = np.ascontiguousarray(maskTh.reshape(CCH, P, G).transpose(1, 0, 2))
    maskBh = np.zeros((P, CCH, P), np.float32)
    for cc in range(CCH):
        for p in range(P):
            maskBh[grp[cc * P + p], cc, p] = 1.0

    shared = dict(
        wqkv8=wqkv8, wpT=wpT, qb=qb, kb=kb, vb=vb, pb=pbh,
        maskT=maskTh, maskB=maskBh,
    )
    in_maps = []
    for b in range(B):
        m = dict(shared)
        m["x"] = np.ascontiguousarray(x[b].reshape(C, N))
        in_maps.append(m)
    return in_maps


def _get_program():
    global _CACHED
    if _CACHED is None:
        _CACHED = build_program()
    return _CACHED


def run(inputs, trace=False, **kw):
    nc = _get_program()
    in_maps = _host_prep(inputs)
    res = bass_utils.run_bass_kernel_spmd(
        nc, in_maps, core_ids=list(range(B)), trace=trace, **kw
    )
    outs = np.stack([res.results[b]["out"].reshape(C, H, W) for b in range(B)])
    return outs.astype(np.float32), res


def kernel(**inputs) -> np.ndarray:
    outs, _ = run(inputs, trace=False)
    return outs


if __name__ == "__main__":
    import reference

    inputs = reference.setup_inputs()
    outs, res = run({k: np.asarray(v) for k, v in inputs.items()}, trace=True)
    print("exec_time_ns:", res.exec_time_ns)


# revision 20
# speedup vs baseline: 1.7719x; 1.0836x over previous
"""Trainium2 Bass kernel for nn_AttentionBlock (B=8, C=512, H=W=64, GN32 + QKV + attention + proj + residual).

Sharding: data-parallel over batch across the 8 NeuronCores (one sample per core).

Per-core pipeline (sample x_b [512, 4096]):
  A. GroupNorm stats: per-channel sum (DVE reduce) and sumsq (ScalarE Square
     with accumulate), group-reduce + per-channel broadcast via mask matmuls.
  B. xn = rs*x + tb (fp8); q/k/vT = 1x1 convs as fp8 DoubleRow matmuls
     (gn gamma/beta and the qk scale C^-0.25 folded into weights on host).
  C. Flash-style attention per 512-wide query block, all fp8 DoubleRow:
       S^T[m,nb] = k^T q ;  P^T = exp(S^T)  (no max subtraction: |S| <~ 1.2)
       O += vT^T P^T ; rowsum += ones^T P^T  (accumulated over m-tile pairs)
       proj on unnormalized O (fp32r), then out = (Wp O)*inv + pb + x
     (softmax normalization commutes with the 1x1 conv, so the reciprocal
      stays off the PE critical path).
"""

import numpy as np
import ml_dtypes

import concourse.bass as bass
import concourse.tile as tile
from concourse import bacc, mybir
from concourse import bass_utils

B = 8
C = 512
H = W = 64
N = H * W          # 4096
G = 32             # groups
CPG = C // G       # 16 channels per group
EPS = 1e-5
P = 128
CCH = C // P       # 4 channel chunks
NBS = 8            # n blocks
NBW = N // NBS     # 512 block width
MTS = N // P       # 32 m tiles

FP32 = mybir.dt.float32
FP32R = mybir.dt.float32r
FP8 = mybir.dt.float8e4

_CACHED = None


def build_program(with_qk_bias=False):
    nc = bacc.Bacc("TRN2", target_bir_lowering=False, debug=False, num_devices=B)

    x = nc.dram_tensor("x", [C, N], FP32, kind="ExternalInput").ap()
    wqkv8 = nc.dram_tensor("wqkv8", [P, CCH, 3 * C], FP8, kind="ExternalInput").ap()
    wpT = nc.dram_tensor("wpT", [P, CCH, C], FP32R, kind="ExternalInput").ap()
    qb = nc.dram_tensor("qb", [P, CCH], FP32, kind="ExternalInput").ap()
    kb = nc.dram_tensor("kb", [P, CCH], FP32, kind="ExternalInput").ap()
    pb = nc.dram_tensor("pb", [P, CCH], FP32, kind="ExternalInput").ap()
    maskT = nc.dram_tensor("maskT", [P, CCH, G], FP32, kind="ExternalInput").ap()
    maskB = nc.dram_tensor("maskB", [P, CCH, P], FP32, kind="ExternalInput").ap()
    out = nc.dram_tensor("out", [C, N], FP32, kind="ExternalOutput").ap()

    with tile.TileContext(nc) as tc:
        _body(nc, tc, x, wqkv8, wpT, qb, kb, pb, maskT, maskB, out, with_qk_bias)
    nc.compile()
    return nc


def _body(nc, tc, x, wqkv8, wpT, qb, kb, pb, maskT, maskB, out, with_qk_bias):
    FT = mybir.ActivationFunctionType
    OP = mybir.AluOpType
    DR = mybir.MatmulPerfMode.DoubleRow

    with (
        tc.tile_pool(name="singles", bufs=1) as singles,
        tc.tile_pool(name="acts", bufs=1) as acts,
    ):
        # ---- constants / weights resident in SBUF ----
        wq_sb = singles.tile([P, CCH, 3 * C], FP8)
        nc.sync.dma_start(wq_sb[:], wqkv8[:])
        wp_sb = singles.tile([P, CCH, C], FP32R)
        nc.sync.dma_start(wp_sb[:], wpT[:])
        if with_qk_bias:
            qb_sb = singles.tile([P, CCH], FP32)
            nc.sync.dma_start(qb_sb[:], qb[:])
            kb_sb = singles.tile([P, CCH], FP32)
            nc.sync.dma_start(kb_sb[:], kb[:])
        pb_sb = singles.tile([P, CCH], FP32)
        nc.sync.dma_start(pb_sb[:], pb[:])
        maskT_sb = singles.tile([P, CCH, G], FP32)
        nc.sync.dma_start(maskT_sb[:], maskT[:])
        maskB_sb = singles.tile([P, CCH, P], FP32)
        nc.sync.dma_start(maskB_sb[:], maskB[:])
        ones_sb = singles.tile([P, 2, P], FP8)
        nc.vector.memset(ones_sb[:], 1.0)
        eps_sb = singles.tile([P, 1], FP32)
        nc.vector.memset(eps_sb[:], EPS)

        # persistent activations
        x_t = [acts.tile([P, N], FP32, name=f"x_{cc}") for cc in range(CCH)]
        xn8 = acts.tile([P, CCH, N], FP8)    # normalized input
        q8 = acts.tile([P, CCH, N], FP8)     # [c'P, c'chunk, n]
        k8 = acts.tile([P, CCH, N], FP8)     # [c'P, c'chunk, m]
        vT8 = acts.tile([P, MTS, C], FP8)    # [mP, mt, c]

        with (
            tc.tile_pool(name="stat", bufs=2) as stat_pool,
            tc.tile_pool(name="gsmall", bufs=1) as gs_pool,
            tc.tile_pool(name="pstat", bufs=2, space="PSUM") as pstat_pool,
        ):
            # ---- Phase A: group norm statistics ----
            # per-channel sum via DVE reduce; sumsq via ScalarE Square+accum
            # (Square main output scratched into q8, overwritten in phase B).
            rhs_stats = gs_pool.tile([P, CCH, 2], FP32)
            for cc in range(CCH):
                nc.sync.dma_start(x_t[cc][:], x[cc * P:(cc + 1) * P, :])
                nc.vector.reduce_sum(
                    rhs_stats[:, cc, 0:1], x_t[cc][:], axis=mybir.AxisListType.X
                )
                nc.scalar.activation(
                    out=q8[:, cc, :], in_=x_t[cc][:], func=FT.Square,
                    accum_out=rhs_stats[:, cc, 1:2],
                )

            # group reduce (maskT carries 1/(CPG*N)): [32,2] = [mean_g, Ex2_g]
            gps = pstat_pool.tile([G, 2], FP32, name="gps")
            for cc in range(CCH):
                nc.tensor.matmul(
                    gps[:], maskT_sb[:, cc, :], rhs_stats[:, cc, :],
                    start=(cc == 0), stop=(cc == CCH - 1),
                )
            gstats = gs_pool.tile([G, 2], FP32)
            nc.vector.tensor_copy(out=gstats[:], in_=gps[:])

            # rs = rsqrt(var_g + eps); tb = -mean_g * rs   (g-padded [128,2])
            ab_g = gs_pool.tile([P, 2], FP32)
            nc.vector.memset(ab_g[:], 0.0)
            gvar = gs_pool.tile([G, 1], FP32)
            nc.vector.tensor_tensor(gvar[:], gstats[:, 0:1], gstats[:, 0:1], OP.mult)
            nc.vector.tensor_sub(gvar[:], gstats[:, 1:2], gvar[:])
            nc.scalar.activation(
                out=gvar[:], in_=gvar[:], func=FT.Sqrt, bias=eps_sb[0:G, :]
            )
            nc.vector.reciprocal(out=ab_g[0:G, 0:1], in_=gvar[:])
            nc.vector.tensor_tensor(
                ab_g[0:G, 1:2], gstats[:, 0:1], ab_g[0:G, 0:1], OP.mult
            )
            nc.vector.tensor_scalar_mul(ab_g[0:G, 1:2], ab_g[0:G, 1:2], -1.0)

            # broadcast to channels: ab_c[:, cc, :] = maskB[:,cc,:].T @ ab_g
            ab_c = gs_pool.tile([P, CCH, 2], FP32)
            for cc in range(CCH):
                abps = pstat_pool.tile([P, 2], FP32, name="abps")
                nc.tensor.matmul(
                    abps[:], maskB_sb[:, cc, :], ab_g[:], start=True, stop=True
                )
                nc.vector.tensor_copy(out=ab_c[:, cc, :], in_=abps[:])

            # ---- xn = rs*x + tb -> fp8 (DVE/ACT split) ----
            for cc in range(CCH):
                if cc % 2 == 0:
                    nc.vector.tensor_scalar(
                        out=xn8[:, cc, :], in0=x_t[cc][:],
                        scalar1=ab_c[:, cc, 0:1], scalar2=ab_c[:, cc, 1:2],
                        op0=OP.mult, op1=OP.add,
                    )
                else:
                    nc.scalar.activation(
                        out=xn8[:, cc, :], in_=x_t[cc][:], func=FT.Identity,
                        bias=ab_c[:, cc, 1:2], scale=ab_c[:, cc, 0:1],
                    )

        # ---- Phase B: qkv projections (fp8 DoubleRow, K=256 per matmul).
        # v bias is exact-folded into the proj bias on host (Wp@vb), so vT
        # (and with zero biases, q/k too) are pure PSUM->fp8 casts. ----
        with tc.tile_pool(name="pqkv", bufs=3, space="PSUM") as pqkv:
            for oc in range(CCH):
                osl = bass.ts(oc, P)
                for nb in range(NBS):
                    nsl = bass.ts(nb, NBW)
                    ps = pqkv.tile([P, NBW], FP32, name="qk_ps")
                    for h in range(2):
                        nc.tensor.matmul(
                            ps[:],
                            wq_sb[:, 2 * h:2 * h + 2, osl],
                            xn8[:, 2 * h:2 * h + 2, nsl],
                            start=(h == 0), stop=(h == 1), perf_mode=DR,
                        )
                    if with_qk_bias:
                        nc.vector.tensor_scalar_add(
                            q8[:, oc, nsl], ps[:], qb_sb[:, oc:oc + 1]
                        )
                    else:
                        nc.vector.tensor_copy(out=q8[:, oc, nsl], in_=ps[:])
                    ps2 = pqkv.tile([P, NBW], FP32, name="qk_ps")
                    for h in range(2):
                        nc.tensor.matmul(
                            ps2[:],
                            wq_sb[:, 2 * h:2 * h + 2, C + oc * P:C + (oc + 1) * P],
                            xn8[:, 2 * h:2 * h + 2, nsl],
                            start=(h == 0), stop=(h == 1), perf_mode=DR,
                        )
                    if with_qk_bias:
                        nc.scalar.activation(
                            out=k8[:, oc, nsl], in_=ps2[:],
                            func=FT.Identity, bias=kb_sb[:, oc:oc + 1],
                        )
                    else:
                        nc.scalar.copy(out=k8[:, oc, nsl], in_=ps2[:])
            for mt in range(MTS):
                msl = bass.ts(mt, P)
                ps = pqkv.tile([P, C], FP32, name="qk_ps")
                for h in range(2):
                    nc.tensor.matmul(
                        ps[:],
                        xn8[:, 2 * h:2 * h + 2, msl],
                        wq_sb[:, 2 * h:2 * h + 2, 2 * C:3 * C],
                        start=(h == 0), stop=(h == 1), perf_mode=DR,
                    )
                if mt % 2 == 0:
                    nc.vector.tensor_copy(out=vT8[:, mt, :], in_=ps[:])
                else:
                    nc.scalar.copy(out=vT8[:, mt, :], in_=ps[:])

        # ---- Phase C: attention + proj + residual ----
        with (
            tc.tile_pool(name="pt", bufs=3) as pt_pool,
            tc.tile_pool(name="hf", bufs=2) as hf_pool,
            tc.tile_pool(name="ost", bufs=4) as ost_pool,
            tc.tile_pool(name="pacc", bufs=1, space="PSUM") as pacc,
            tc.tile_pool(name="psmall", bufs=1, space="PSUM") as psm,
            tc.tile_pool(name="pproj", bufs=1, space="PSUM") as ppr,
        ):
            for nb in range(NBS):
                nsl = bass.ts(nb, NBW)
                o_ps = pacc.tile([P, CCH, NBW], FP32, name="o_ps")
                r_ps = pacc.tile([P, NBW], FP32, name="r_ps")

                def ov_mms(u, pt):
                    for oc in range(CCH):
                        nc.tensor.matmul(
                            o_ps[:, oc, :],
                            vT8[:, 2 * u:2 * u + 2, bass.ts(oc, P)],
                            pt[:],
                            start=(u == 0), stop=(u == MTS // 2 - 1), perf_mode=DR,
                        )
                    nc.tensor.matmul(
                        r_ps[:], ones_sb[:], pt[:],
                        start=(u == 0), stop=(u == MTS // 2 - 1), perf_mode=DR,
                    )

                # software pipeline: S(u) -> O(u-1) -> exp(u), one PSUM s-pair
                prev = None
                for u in range(MTS // 2):
                    s_ps = psm.tile([P, 2, NBW], FP32, name="s_ps")
                    for j in range(2):
                        msl = bass.ts(2 * u + j, P)
                        for h in range(2):
                            nc.tensor.matmul(
                                s_ps[:, j, :],
                                k8[:, 2 * h:2 * h + 2, msl],
                                q8[:, 2 * h:2 * h + 2, nsl],
                                start=(h == 0), stop=(h == 1), perf_mode=DR,
                            )
                    if prev is not None:
                        ov_mms(u - 1, prev)
                    pt = pt_pool.tile([P, 2, NBW], FP8, name="pt")
                    nc.scalar.activation(out=pt[:], in_=s_ps[:], func=FT.Exp)
                    prev = pt
                ov_mms(MTS // 2 - 1, prev)

                # unnormalized O -> fp32r (ScalarE; DVE is busy at the seam);
                # softmax denominator applied after proj (it commutes).
                inv = hf_pool.tile([P, NBW], FP32, name="inv")
                hfr = hf_pool.tile([P, CCH, NBW], FP32R, name="hfr")
                for oc in range(CCH):
                    nc.scalar.copy(out=hfr[:, oc, :], in_=o_ps[:, oc, :])
                nc.vector.reciprocal_approx_fast(out=inv[:], in_=r_ps[:])
                for oc in range(CCH):
                    pr_ps = ppr.tile([P, NBW], FP32, name="pr_ps")
                    for cc in range(CCH):
                        nc.tensor.matmul(
                            pr_ps[:],
                            wp_sb[:, cc, bass.ts(oc, P)],
                            hfr[:, cc, :],
                            start=(cc == 0), stop=(cc == CCH - 1),
                        )
                    o_t = ost_pool.tile([P, NBW], FP32, name="o_t")
                    nc.vector.tensor_tensor(o_t[:], pr_ps[:], inv[:], OP.mult)
                    nc.vector.scalar_tensor_tensor(
                        out=o_t[:], in0=o_t[:], scalar=pb_sb[:, oc:oc + 1],
                        in1=x_t[oc][:, nsl], op0=OP.add, op1=OP.add,
                    )
                    nc.sync.dma_start(out[oc * P:(oc + 1) * P, nsl], o_t[:])


def _host_prep(inputs):
    x = np.ascontiguousarray(np.asarray(inputs["x"], dtype=np.float32))
    gamma = np.asarray(inputs["gn_gamma"], dtype=np.float32)
    beta = np.asarray(inputs["gn_beta"], dtype=np.float32)
    qkv_w = np.asarray(inputs["qkv_w"], dtype=np.float32)
    qkv_b = np.asarray(inputs["qkv_b"], dtype=np.float32)
    proj_w = np.asarray(inputs["proj_w"], dtype=np.float32)
    proj_b = np.asarray(inputs["proj_b"], dtype=np.float32)

    scale = float(C) ** -0.25
    Wf = (qkv_w * gamma[None, :]).copy()
    bf = (qkv_b + qkv_w @ beta).copy()
    Wf[: 2 * C] *= scale
    bf[: 2 * C] *= scale

    wqkv8 = np.ascontiguousarray(
        Wf.T.reshape(CCH, P, 3 * C).transpose(1, 0, 2)
    ).astype(ml_dtypes.float8_e4m3fn)
    wpT = np.ascontiguousarray(
        proj_w.T.reshape(CCH, P, C).transpose(1, 0, 2), dtype=np.float32
    )
    qb = np.ascontiguousarray(bf[0:C].reshape(CCH, P).T, dtype=np.float32)
    kb = np.ascontiguousarray(bf[C:2 * C].reshape(CCH, P).T, dtype=np.float32)
    # v bias folds exactly through attention + proj: proj(O + vb 1^T) =
    # proj(O) + (Wp vb) 1^T  (softmax rows sum to 1)
    pb_eff = proj_b + proj_w @ bf[2 * C:3 * C]
    pbh = np.ascontiguousarray(pb_eff.reshape(CCH, P).T, dtype=np.float32)
    with_qk_bias = bool(np.any(bf[0:2 * C] != 0.0))

    chan = np.arange(C)
    grp = chan // CPG
    maskTh = np.zeros((C, G), np.float32)
    maskTh[chan, grp] = 1.0 / (CPG * N)
    maskTh = np.ascontiguousarray(maskTh.reshape(CCH, P, G).transpose(1, 0, 2))
    maskBh = np.zeros((P, CCH, P), np.float32)
    for cc in range(CCH):
        for p in range(P):
            maskBh[grp[cc * P + p], cc, p] = 1.0

    shared = dict(
        wqkv8=wqkv8, wpT=wpT, qb=qb, kb=kb, pb=pbh,
        maskT=maskTh, maskB=maskBh,
    )
    in_maps = []
    for b in range(B):
        m = dict(shared)
        m["x"] = np.ascontiguousarray(x[b].reshape(C, N))
        in_maps.append(m)
    return in_maps, with_qk_bias


def _get_program(with_qk_bias):
    global _CACHED
    if _CACHED is None or _CACHED[0] != with_qk_bias:
        _CACHED = (with_qk_bias, build_program(with_qk_bias))
    return _CACHED[1]


def run(inputs, trace=False, **kw):
    in_maps, with_qk_bias = _host_prep(inputs)
    nc = _get_program(with_qk_bias)
    res = bass_utils.run_bass_kernel_spmd(
        nc, in_maps, core_ids=list(range(B)), trace=trace, **kw
    )
    outs = np.stack([res.results[b]["out"].reshape(C, H, W) for b in range(B)])
    return outs.astype(np.float32), res


def kernel(**inputs) -> np.ndarray:
    outs, _ = run(inputs, trace=False)
    return outs


if __name__ == "__main__":
    import reference

    inputs = reference.setup_inputs()
    outs, res = run({k: np.asarray(v) for k, v in inputs.items()}, trace=True)
    print("exec_time_ns:", res.exec_time_ns)
